# revision 74
# baseline (speedup 1.0000x reference)
"""Trainium2 Bass kernel for DisentangleStaticNoiseLoss (NT-Xent style loss).

Math (matches the jax reference):
    x   : [K=8192, D=128] stacked embeddings (N=8 blocks of BS=1024)
    z   : row-normalized x;  S = (z @ z.T) / 0.5
    row i (block b, sample r): positives = S[i, r + b'*BS] for b' != b,
    negatives = all j with j % BS != r.
    loss = mean over (i, pos) of [log(exp(pos) + sum_neg exp(neg)) - pos]

Sharding (exploits S symmetry): core c owns the 1024 rows of block c but
computes only local column blocks k=0..4 (global blocks c..c+4 mod 8), i.e.
5/8 of the columns. Every unordered block pair is covered exactly once
(k=1,2,3 pairs by the lower core; k=4 pairs twice -- cheap redundancy that
keeps the SPMD program identical on all cores). Per (k, m-tile):
  - 2 bf16 matmuls into PSUM, one ACT Exp -> esc bf16 in SBUF, with the
    row sums (own-row F partials) accumulated for free via ACT accum_out
  - positive diagonals via DVE masked reduce
  - column sums for k=1,2,3 (F partials for the mirrored rows, by symmetry
    exp(S)^T = exp(S)) via ones-vector matmuls accumulated in PSUM over m
The host pre-shards the input (per-core rotation + row-normalize + bf16 +
transpose -- the "all-gather z" of the sharding hint, 0.01% of the FLOPs),
then assembles F from the row/col partials, the positive logits from the
diagonals + their mirrors, and does the tiny [8192 x 8] logsumexp finale in
float64 (the final all-reduce of the hint).
"""

import sys

import numpy as np

if "/opt/trn_rl_repo" not in sys.path:
    sys.path.insert(0, "/opt/trn_rl_repo")

N = 8
BS = 1024
D = 128
K = N * BS          # 8192
NCORES = 8
ROWS = K // NCORES  # 1024 rows per core
MT = ROWS // 128    # 8 m-tiles of 128 rows
KB = 5              # column blocks computed per core (k = 0..4)
CW = 1024           # supertile column width = one block
TEMP_SCALE = 2.0    # 1 / temperature
EPS = 1e-8

_NC_CACHE = {}


def _build_nc():
    import concourse.bacc as bacc
    import concourse.bass as bass
    import concourse.tile as tile
    from concourse import mybir

    f32 = mybir.dt.float32
    bf16 = mybir.dt.bfloat16
    fp8 = mybir.dt.float8e4
    OP = mybir.AluOpType
    AF = mybir.ActivationFunctionType
    PMODE = mybir.MatmulPerfMode

    nc = bacc.Bacc("TRN2", target_bir_lowering=False, debug=False)
    zin = nc.declare_dram_parameter("zin", [128, KB * BS], bf16, isOutput=False)
    ident = nc.declare_dram_parameter("ident", [128, 128], f32, isOutput=False)
    frow_o = nc.declare_dram_parameter("frow_out", [128, KB * MT], f32, isOutput=True)
    sdiag_o = nc.declare_dram_parameter("sdiag_out", [128, KB * MT], f32, isOutput=True)
    csum_o = nc.declare_dram_parameter("csum_out", [4, CW], f32, isOutput=True)
    pk4_o = nc.declare_dram_parameter("pk4_out", [128, 16], f32, isOutput=True)

    with tile.TileContext(nc) as tc:
        with (
            tc.tile_pool(name="persist", bufs=1) as P,
            tc.tile_pool(name="work", bufs=3) as W,
        ):
            zT = P.tile([128, KB * BS], bf16, tag="zT")   # z transposed: [D, 5120]
            idsb = P.tile([128, 128], f32, tag="idsb")    # identity f32
            idsb8 = P.tile([128, 128], fp8, tag="idsb8")
            ones8 = P.tile([128, 2, 128], fp8, tag="ones8")
            nones8 = P.tile([128, 128], fp8, tag="nones8")  # -1s (T subtract)
            wsrc = P.tile([128, 512], bf16, tag="wsrc")   # PE warm-up source
            frow = P.tile([128, KB * MT], f32, tag="frow")    # row-sum partials
            pk4 = P.tile([128, 16], f32, tag="pk4")  # packed k4 frow|sdiag
            sdiag = P.tile([128, KB * MT], f32, tag="sdiag")  # exp(pos) diagonals
            csbs = [
                P.tile([1, CW], f32, tag=f"csb{j}", name=f"csb{j}")
                for j in range(4)
            ]  # col sums staged in SBUF (partition 0 each); row 0 = k0
            #    triangle csC - T, rows 1..3 = k1..k3 full col sums

            # Preload the Exp table at t~0 so the main loop never waits on it.
            dum = P.tile([128, 1], f32, tag="dum")
            nc.vector.memset(dum[:], 0.0)
            nc.scalar.activation(out=dum[:], in_=dum[:], func=AF.Exp)

            # split the z load so block 0 lands first and matmuls start
            # early: its two halves go on independent DGE rings (SP + ACT)
            # to process descriptors in parallel, and the bulk goes on the
            # software-DGE ring so its descriptors don't delay block 0's
            nc.sync.dma_start(out=zT[:, 0:512], in_=zin[:, 0:512])
            nc.scalar.dma_start(out=zT[:, 512:CW], in_=zin[:, 512:CW])
            nc.gpsimd.dma_start(out=zT[:, CW:], in_=zin[:, CW:])
            nc.sync.dma_start(out=idsb[:], in_=ident[:, :])
            nc.vector.memset(ones8[:], 1.0)
            nc.vector.memset(nones8[:], -1.0)
            nc.vector.memset(wsrc[:], 0.0)
            nc.vector.tensor_copy(out=idsb8[:], in_=idsb[:])

            # ---- main loop: S block, exp, row sums, diagonals, col sums ---
            with (
                tc.tile_pool(name="pmm", bufs=2, space="PSUM") as PM,
                tc.tile_pool(name="pcs", bufs=2, space="PSUM") as PC,
            ):
                # warm the PE through its p-state ramp while zT streams in,
                # so the first real matmuls run at full clock
                for w in range(8):
                    wps = PM.tile([128, 512], f32, tag="ps", name="wps")
                    nc.tensor.matmul(
                        wps[:, 0:512], wsrc[:, 0:128], wsrc[:], start=True, stop=True
                    )

                cs_tiles = {}
                pending = []  # deferred col-sum matmuls: (kind, ...)

                def flush_pending(drain_to=0):
                    while len(pending) > drain_to:
                        item = pending.pop(0)
                        kk = item[0]
                        if kk not in cs_tiles:
                            cs_tiles[kk] = PC.tile(
                                [1, CW], f32, tag="cs", name=f"cs{kk}"
                            )
                        cs = cs_tiles[kk]
                        if kk == 0:
                            # k0 triangle: per m-tile t, col sums of the
                            # computed cols [t*128, 1024) plus a negated
                            # same-tile-block term; ragged plain fp8 matmuls
                            _, t, e = item
                            base = t * 128
                            pos = base
                            while pos < CW:
                                nxt = min((pos // 512 + 1) * 512, CW)
                                nc.tensor.matmul(
                                    cs[:, pos:nxt],
                                    ones8[:, 0, 0:1],
                                    e[:, pos - base : nxt - base],
                                    start=(t == 0),
                                    stop=(t == MT - 1),
                                    skip_group_check=True,
                                )
                                pos = nxt
                            nc.tensor.matmul(
                                cs[:, base : base + 128],
                                nones8[:, 0:1],
                                e[:, 0:128],
                                start=False,
                                stop=False,
                                skip_group_check=True,
                            )
                            if t == MT - 1:
                                nc.vector.tensor_copy(
                                    out=csbs[0][:], in_=cs[:]
                                )
                                nc.sync.dma_start(
                                    out=csum_o[0:1, :], in_=csbs[0][:]
                                )
                        else:
                            _, q, e = item
                            for h in range(2):
                                nc.tensor.matmul(
                                    cs[:, h * 512 : (h + 1) * 512],
                                    ones8[:, :, 0:1],
                                    e[:, 2 * q : 2 * q + 2, h * 512 : (h + 1) * 512],
                                    start=(q == 0),
                                    stop=(q == MT // 2 - 1),
                                    perf_mode=PMODE.DoubleRow,
                                )
                            if q == MT // 2 - 1:
                                nc.vector.tensor_copy(
                                    out=csbs[kk][:], in_=cs[:]
                                )
                                nc.sync.dma_start(
                                    out=csum_o[kk : kk + 1, :], in_=csbs[kk][:]
                                )

                # k4 (no col sums) right after the PE-heavy k0 triangle so
                # its idle PE absorbs the deferred k0 col-sum matmuls; the
                # packed tail tile then captures the last phase, k3
                for k in (0, KB - 1, 1, 2, 3):
                    need_cs = k in (1, 2, 3)
                    esck = None
                    for m in range(MT):
                        tri = k == 0
                        width = CW - m * 128 if tri else CW
                        cbase = k * CW + (m * 128 if tri else 0)
                        ps = PM.tile([128, CW], f32, tag="ps")
                        lhsT = zT[:, m * 128 : (m + 1) * 128]
                        off = 0
                        while off < width:
                            w = min(512, width - off)
                            nc.tensor.matmul(
                                ps[:, off : off + w],
                                lhsT,
                                zT[:, cbase + off : cbase + off + w],
                                start=True,
                                stop=True,
                            )
                            off += w
                        # col-sum matmuls of previous slots go here so the
                        # PE never waits on the ACT output it consumes;
                        # keep up to 2 queued to smooth PE load across phases
                        flush_pending(drain_to=2)
                        if need_cs and esck is None:
                            esck = W.tile(
                                [128, MT, CW], fp8, tag="esck", bufs=2, name="esck"
                            )
                        col = k * MT + m
                        eout = (
                            esck[:, m, :]
                            if need_cs
                            else W.tile([128, CW], fp8, tag="esc0", bufs=4)
                        )
                        # last-phase (k3) partials accumulate straight into
                        # the packed output tile: no copies on the tail path
                        facc = (
                            pk4[:, m : m + 1]
                            if k == 3
                            else frow[:, col : col + 1]
                        )
                        nc.scalar.activation(
                            out=eout[:, 0:width],
                            in_=ps[:, 0:width],
                            func=AF.Exp,
                            scale=TEMP_SCALE,
                            accum_out=facc,
                        )
                        # positive diagonal (DVE identity mask on fp8 esc;
                        # reading SBUF keeps the PSUM tile lifetime short).
                        # For the k0 triangle the diag block sits at offset 0.
                        doff = 0 if tri else m * 128
                        sacc = (
                            pk4[:, 8 + m : 9 + m]
                            if k == 3
                            else sdiag[:, col : col + 1]
                        )
                        dscr = W.tile([128, 128], fp8, tag="dscr", bufs=2)
                        nc.vector.scalar_tensor_tensor(
                            out=dscr[:],
                            in0=eout[:, doff : doff + 128],
                            scalar=1.0,
                            in1=idsb8[:],
                            op0=OP.mult,
                            op1=OP.mult,
                            accum_out=sacc,
                        )
                        if tri:
                            pending.append((0, m, eout))
                        elif need_cs and m % 2 == 1:
                            pending.append((k, m // 2, esck))
                        if k == 3 and m == 0:
                            # k0, k1, k2, k4 partials are final: ship early
                            nc.sync.dma_start(
                                out=frow_o[:, 0 : 3 * MT],
                                in_=frow[:, 0 : 3 * MT],
                            )
                            nc.sync.dma_start(
                                out=sdiag_o[:, 0 : 3 * MT],
                                in_=sdiag[:, 0 : 3 * MT],
                            )
                            nc.sync.dma_start(
                                out=frow_o[:, 4 * MT :],
                                in_=frow[:, 4 * MT :],
                            )
                            nc.sync.dma_start(
                                out=sdiag_o[:, 4 * MT :],
                                in_=sdiag[:, 4 * MT :],
                            )
                flush_pending()

                nc.scalar.dma_start(out=pk4_o[:, :], in_=pk4[:])

    nc.compile()
    return nc


def _get_nc():
    if "nc" not in _NC_CACHE:
        _NC_CACHE["nc"] = _build_nc()
    return _NC_CACHE["nc"]


def _make_in_maps(x):
    """Host-side shard prep: normalize rows (the cosine-similarity z),
    cast bf16, transpose to [D, K], and hand each core its rotated
    5-block slice (the 'all-gather z' of the sharding hint)."""
    import ml_dtypes

    ident = np.eye(128, dtype=np.float32)
    nrm = np.maximum(np.sqrt((x.astype(np.float64) ** 2).sum(axis=1)), EPS)
    z = (x / nrm[:, None].astype(np.float32)).astype(ml_dtypes.bfloat16)
    zTT = np.concatenate([z.T, z.T[:, : (KB - 1) * BS]], axis=1)  # [128, 12288]
    in_maps = []
    for c in range(NCORES):
        zc = np.ascontiguousarray(zTT[:, c * BS : c * BS + KB * BS])
        in_maps.append({"zin": zc, "ident": ident})
    return in_maps


def _host_finale(results):
    """Assemble F, positive diagonals, and do the logsumexp finale (f64)."""
    F = np.zeros(K, dtype=np.float64)
    gexp = np.zeros((K, N), dtype=np.float64)
    l_pm = np.arange(8)[None, :] * 128 + np.arange(128)[:, None]  # [p, m]
    for c in range(NCORES):
        r = results[c]
        pk = np.asarray(r["pk4_out"], dtype=np.float64)
        fr = np.asarray(r["frow_out"], dtype=np.float64).copy()
        sd = np.asarray(r["sdiag_out"], dtype=np.float64).copy()
        fr[:, 3 * MT : 4 * MT] = pk[:, 0:8]
        sd[:, 3 * MT : 4 * MT] = pk[:, 8:16]
        fr = fr.reshape(128, KB, MT)
        sd = sd.reshape(128, KB, MT)
        cs = np.asarray(r["csum_out"], dtype=np.float64)
        gi = c * BS + l_pm  # [p, m] global row
        F[gi] += fr.sum(axis=1)
        # k0 is computed as an upper triangle; row 0 of csum carries the
        # below-diagonal part of each own row (strict lower-tile col sums)
        F[c * BS : (c + 1) * BS] += cs[0]
        for k in range(KB):
            d = (c + k) % N
            gexp[gi, d] = sd[:, k, :]
            gexp[d * BS + l_pm, c] = sd[:, k, :]  # mirror (S symmetric)
        for k in (1, 2, 3):
            d = (c + k) % N
            F[d * BS : (d + 1) * BS] += cs[k]
    P = gexp.sum(axis=1)
    A = F - P
    b = np.arange(K) // BS
    g = np.log(gexp)
    L = np.log(gexp + A[:, None]) - g
    L[np.arange(K), b] = 0.0
    loss = L.sum() / (K * (N - 1))
    return np.float32(loss)


def kernel(sim: np.ndarray, _want_results: bool = False, _trace: bool = False):
    x = np.ascontiguousarray(np.asarray(sim, dtype=np.float32).reshape(K, D))
    in_maps = _make_in_maps(x)
    nc = _get_nc()
    from concourse.bass_utils import run_bass_kernel_spmd

    res = run_bass_kernel_spmd(nc, in_maps, list(range(NCORES)), trace=_trace)
    loss = _host_finale(res.results)
    if _want_results:
        return loss, res
    return loss


if __name__ == "__main__":
    nc = _build_nc()
    print("build OK")
